# revision 6
# baseline (speedup 1.0000x reference)
"""Causal self-attention (B=4, T=2048, C=1024, H=16) on 8 TRN2 NeuronCores.

Sharding: core = (batch b, head-group g) with b in 0..3, g in 0..1.
Each core handles one batch element and 8 of the 16 heads (tensor-parallel
split of the QKV / proj weights).  Each core produces one partial [T, C]
output (c_proj contracted over its 8 heads, accumulated in PSUM across the
4 head pairs); the host sums the two per-batch partials and adds b_proj.

Device layout (per core) -- everything SBUF-resident, bf16 matmul inputs:
  xT    [C, T]        x[b]^T, host-transposed + bf16-cast
  wqkv  [C, 3*CL]     W_attn column slice for this head group (q scaled by
                      1/sqrt(D) on host), bf16
  qkvT = wqkv.T @ xT computed as [ch, t] tiles (q^T, k^T); v computed in
  natural [t, d] orientation as xT.T @ wv.
  Attention per head pair: S^T[j, i] = k^T.T q^T (contraction d=64, two
  heads packed on PE row-groups 0-63 / 64-127 running concurrently),
  structural causal masking (only lower-triangular j-tiles computed;
  diagonal tiles get an additive -1e9 triangle constant), exp on ScalarE
  (no max subtraction -- scores are O(6)), P^T @ V' on PE with
  V' = [V | ones] so row 64 of the accumulator is the softmax denominator.

  Schedule is query-chunk-outer (it = chunk of 512 queries): for each it,
  the 4 head pairs run their attention for that chunk; c_proj for chunk
  it-1 (contraction over all 4 pairs, K=512 accumulated in PSUM) drains as
  PE filler between attention slots, together with the next chunk's q/k
  projections and V tiles.  This keeps ScalarE (the exp stream, ~1.15us
  per slot vs ~0.64us of PE work) saturated while the PE runs projection
  matmuls in the gaps, and keeps the PE HAM-warm (no >3us idle windows).

  Normalization: ln(den) read directly from the PSUM accumulator row 64,
  rec = exp(-ln(den)) (Ln and Exp share one ACT table set), DMA of the rec
  row to partition 0, gpsimd partition-broadcast to 64 partitions, then
  one DVE multiply per head; the unnormalized y rows are copied to SBUF
  right after the last PV matmul so the single PSUM accumulator can be
  reused by the next head pair immediately.
"""

import math

import ml_dtypes
import numpy as np

import concourse.bass as bass
import concourse.tile as tile
from concourse import bacc, mybir
from concourse.bass_utils import run_bass_kernel_spmd

# problem shape (hardcoded per the task contract)
B, T, C, H = 4, 2048, 1024, 16
D = C // H            # 64 head dim
NCORES = 8
HL = H // 2           # heads per core
CL = HL * D           # 512 local channels per core
NEG = -1.0e9

P = 128               # SBUF partitions
TI = 512              # query chunk (matmul moving dim)
TJ = 128              # key tile
CT = C // P           # 8 contraction tiles for the projections
NTT = T // P          # 16 t-tiles of 128
NIT = T // TI         # 4 query chunks
JQ = CL // P          # 4 channel tiles for q (and for k, and for y)
KC = CL // P          # 4 channel tiles in c_proj contraction
NOC = C // TI         # 2 output-column tiles in c_proj

FP32 = mybir.dt.float32
BF16 = mybir.dt.bfloat16
AF = mybir.ActivationFunctionType
ADD = mybir.AluOpType.add
MULT = mybir.AluOpType.mult


def _emit(tc, io):
    nc = tc.nc
    xT, wqkv, bqk, bv, wp, mtri, out, out2 = (
        io["xT"], io["wqkv"], io["bqk"], io["bv"], io["wp"], io["mtri"],
        io["out"], io["out2"],
    )

    with (
        tc.tile_pool(name="const", bufs=1) as cpool,
        tc.tile_pool(name="work", bufs=4) as wpool,
        tc.tile_pool(name="epi", bufs=2) as epool,
        tc.tile_pool(name="outp", bufs=3) as opool,
        tc.tile_pool(name="ps", bufs=2, space="PSUM") as s_ps,
        tc.tile_pool(name="po", bufs=1, space="PSUM") as o_ps,
        tc.tile_pool(name="mm", bufs=1, space="PSUM") as mm_ps,
        tc.tile_pool(name="cp", bufs=1, space="PSUM") as cp_ps,
    ):
        # persistent SBUF tensors
        xT_sb = cpool.tile([P, CT, T], BF16)
        wqkv_sb = cpool.tile([P, CT, 3 * CL], BF16)
        qT_sb = cpool.tile([P, JQ, T], BF16)
        kT_sb = cpool.tile([P, JQ, T], BF16)
        v_sb = cpool.tile([P, NTT, HL, D + 1], BF16)
        yT_sb = cpool.tile([P, JQ, T], BF16)
        wp_sb = cpool.tile([P, KC, C], BF16)
        mtri_sb = cpool.tile([P, P], FP32)
        bqk_sb = cpool.tile([P, 2 * JQ], FP32)
        bv_sb = cpool.tile([1, CL], FP32)
        bvb_sb = cpool.tile([P, CL], FP32)

        xT_d = xT.rearrange("(o p) t -> p o t", p=P)
        wqkv_d = wqkv.rearrange("(o p) j -> p o j", p=P)

        # prologue DMAs in first-compute order: tiny constants, then the
        # pair-0 q/k weight column slices + x^T chunk 0 (first matmuls),
        # v weights, then the rest interleaved so each pair / chunk lands
        # just before its first use.  Total input is ~8.4MB = ~23us at HBM
        # rate; fine-grained ordering lets compute start at ~4us.
        # first-needed loads on the sync HWDGE ring (fast dispatch); the
        # rest on the gpsimd SWDGE ring so the two drain in parallel and
        # the sync ring stays clear for the epilogue row DMAs
        def dma_wslice(eng, which, pr):
            j0 = which * CL + pr * P
            eng.dma_start(wqkv_sb[:, :, j0 : j0 + P], wqkv_d[:, :, j0 : j0 + P])

        nc.sync.dma_start(xT_sb[:, :, 0:TI], xT_d[:, :, 0:TI])
        dma_wslice(nc.sync, 1, 0)  # w_k pair 0
        dma_wslice(nc.sync, 0, 0)  # w_q pair 0
        nc.sync.dma_start(bqk_sb[:], bqk[:])
        nc.sync.dma_start(bv_sb[:], bv[:])
        nc.sync.dma_start(mtri_sb[:], mtri[:])
        nc.sync.dma_start(wqkv_sb[:, :, 2 * CL :], wqkv_d[:, :, 2 * CL :])  # w_v
        nc.gpsimd.partition_broadcast(bvb_sb[:], bv_sb[:])
        dma_wslice(nc.gpsimd, 1, 1)
        dma_wslice(nc.gpsimd, 0, 1)
        nc.gpsimd.dma_start(xT_sb[:, :, TI : 2 * TI], xT_d[:, :, TI : 2 * TI])
        dma_wslice(nc.gpsimd, 1, 2)
        dma_wslice(nc.gpsimd, 0, 2)
        dma_wslice(nc.gpsimd, 1, 3)
        dma_wslice(nc.gpsimd, 0, 3)
        nc.gpsimd.dma_start(xT_sb[:, :, 2 * TI : 3 * TI], xT_d[:, :, 2 * TI : 3 * TI])
        nc.gpsimd.dma_start(wp_sb[:], wp.rearrange("(o p) j -> p o j", p=P))
        nc.gpsimd.dma_start(xT_sb[:, :, 3 * TI :], xT_d[:, :, 3 * TI :])
        # ones column of V' (softmax denominator accumulator)
        nc.vector.memset(v_sb[:, :, :, D : D + 1], 1.0)

        wv = wqkv_sb[:, :, 2 * CL : 3 * CL]

        # ---- PE filler: projection / c_proj work queued as ~1-matmul
        # items and drained between attention slots.  Items carry a key on
        # their last (finalizing) op so attention slots can force-drain
        # their producers before being emitted -- the Tile framework only
        # tracks dependencies in emission order, so a consumer emitted
        # before its producer would silently read stale SBUF. ----
        queue = []
        done = set()
        slots_left = [sum(4 * (it + 1) for it in range(NIT)) * JQ + 4 * NIT]

        def drain(n):
            for _ in range(min(n, len(queue))):
                key, f = queue.pop(0)
                f()
                if key is not None:
                    done.add(key)

        def drain_until(key):
            while key not in done:
                assert queue, f"filler item {key} was never enqueued"
                k, f = queue.pop(0)
                f()
                if k is not None:
                    done.add(k)

        def v_tile_items(tt):
            """V in natural [t, d] orientation: V = xT.T @ wv, one t-tile."""
            state = {}

            def mk(o):
                def f():
                    if o == 0:
                        state["t"] = mm_ps.tile([P, CL], FP32, tag="mm", name="vmm")
                    nc.tensor.matmul(
                        state["t"][:],
                        xT_sb[:, o, tt * P : (tt + 1) * P],
                        wv[:, o, :],
                        start=(o == 0),
                        stop=(o == CT - 1),
                    )
                return f

            items = [(None, mk(o)) for o in range(CT)]

            def bias():
                nc.vector.tensor_tensor(
                    v_sb[:, tt, :, 0:D],
                    state["t"].rearrange("p (h d) -> p h d", h=HL),
                    bvb_sb.rearrange("p (h d) -> p h d", h=HL),
                    ADD,
                )

            items.append((("v", tt), bias))
            return items

        def qkv_group_items(pr, which, tch):
            """One [128-ch, 512-t] q^T (which=0) or k^T (which=1) tile."""
            jt = which * JQ + pr
            dst = qT_sb if which == 0 else kT_sb
            state = {}

            def mk(o):
                def f():
                    if o == 0:
                        state["t"] = mm_ps.tile([P, TI], FP32, tag="mm", name="qkmm")
                    nc.tensor.matmul(
                        state["t"][:],
                        wqkv_sb[:, o, jt * P : (jt + 1) * P],
                        xT_sb[:, o, tch * TI : (tch + 1) * TI],
                        start=(o == 0),
                        stop=(o == CT - 1),
                    )
                return f

            items = [(None, mk(o)) for o in range(CT)]

            def bias():
                nc.vector.tensor_scalar_add(
                    dst[:, pr, tch * TI : (tch + 1) * TI],
                    state["t"][:],
                    bqk_sb[:, jt : jt + 1],
                )

            items.append((("qkv", which, pr, tch), bias))
            return items

        def cproj_items(tt, oc):
            """One [128-t, 512-c] c_proj output tile, K=512 accumulated in
            PSUM over the 4 head pairs, stored to DRAM as bf16."""
            state = {}

            def mk(pr):
                def f():
                    if pr == 0:
                        state["t"] = cp_ps.tile([P, TI], FP32, tag="cp", name="cpmm")
                    nc.tensor.matmul(
                        state["t"][:],
                        yT_sb[:, pr, tt * P : (tt + 1) * P],
                        wp_sb[:, pr, oc * TI : (oc + 1) * TI],
                        start=(pr == 0),
                        stop=(pr == JQ - 1),
                    )
                return f

            items = [(None, mk(pr)) for pr in range(JQ)]

            def store():
                ob = opool.tile([P, TI], BF16, tag="ob", name="ob")
                nc.vector.tensor_copy(ob[:], state["t"][:])
                nc.sync.dma_start(
                    out[tt * P : (tt + 1) * P, oc * TI : (oc + 1) * TI], ob[:]
                )

            items.append((None, store))
            return items

        # ---- attention slot + unit epilogue ----
        def scores_part(pr, it, jt):
            # force-emit this slot's producers (emission order = the only
            # dependency order Tile sees)
            drain_until(("qkv", 0, pr, it))
            drain_until(("qkv", 1, pr, jt * TJ // TI))
            delta = jt * TJ - it * TI
            lo = max(delta, 0)
            ps = s_ps.tile([P, 2, TI], FP32, tag="ps")
            # S^T = k^T.T @ q^T, contraction d=64; the two heads of the
            # pair sit on PE row groups 0-63 / 64-127 and run concurrently.
            nc.tensor.matmul(
                ps[:, 0, lo:TI],
                kT_sb[0:D, pr, jt * TJ : (jt + 1) * TJ],
                qT_sb[0:D, pr, it * TI + lo : (it + 1) * TI],
                start=True,
                stop=True,
            )
            nc.tensor.matmul(
                ps[:, 1, lo:TI],
                kT_sb[D:P, pr, jt * TJ : (jt + 1) * TJ],
                qT_sb[D:P, pr, it * TI + lo : (it + 1) * TI],
                start=True,
                stop=True,
                tile_position=(D, 0),
            )
            if delta >= 0:  # diagonal tile: strict upper triangle -> -1e9
                nc.vector.tensor_tensor(
                    ps[:, :, delta : delta + TJ],
                    ps[:, :, delta : delta + TJ],
                    mtri_sb[:, None, :].to_broadcast((P, 2, TJ)),
                    ADD,
                )
            p2 = wpool.tile([P, 2, TI], BF16, tag="p2")
            # columns [0:lo) are fully masked and the PV matmuls only read
            # [lo:], so exp is restricted and no memset is needed
            if lo > 0:
                nc.scalar.activation(p2[:, :, lo:TI], ps[:, :, lo:TI], AF.Exp)
            else:
                nc.scalar.activation(p2[:], ps[:], AF.Exp)
            return p2, lo

        def pv_part(pr, jt, njt, po, p2, lo):
            drain_until(("v", jt))
            first, last = (jt == 0), (jt == njt - 1)
            nc.tensor.matmul(
                po[0 : D + 1, 0, lo:TI],
                v_sb[:, jt, 2 * pr, :],
                p2[:, 0, lo:TI],
                start=first,
                stop=last,
            )
            nc.tensor.matmul(
                po[0 : D + 1, 1, lo:TI],
                v_sb[:, jt, 2 * pr + 1, :],
                p2[:, 1, lo:TI],
                start=first,
                stop=last,
            )

        def slot(pr, it, jt, njt, po):
            p2, lo = scores_part(pr, it, jt)
            pv_part(pr, jt, njt, po, p2, lo)

        def epi_copy(po):
            # free the PSUM accumulator fast: plain copy of y-hat + den
            osb = epool.tile([D + 1, 2, TI], FP32, tag="osb")
            nc.vector.tensor_copy(osb[:], po[0 : D + 1, :, :])
            return osb

        def epi_rest(pr, it, po, osb):
            """Normalize the pair's y^T rows for this it-chunk.  Emitted
            after the NEXT unit's first exp so the Ln/Exp pair does not
            stall the ScalarE exp stream at the unit boundary (the Ln
            waits on this unit's last PV matmul)."""
            islice = slice(it * TI, (it + 1) * TI)
            # rec = exp(-ln(den)) -- Ln/Exp share one ACT table set, and
            # Ln reads the denominator row straight from PSUM (both
            # parities in one [1, 1024] pass: the two po banks are
            # adjacent inside the single [128, 2, 512] accumulator tile).
            rl = epool.tile([D + 1, 2, TI], FP32, tag="rl")
            nc.scalar.activation(rl[D : D + 1, :, :], po[D : D + 1, :, :], AF.Ln)
            rc = epool.tile([D + 1, 2, TI], FP32, tag="rc")
            nc.scalar.activation(
                rc[D : D + 1, :, :], rl[D : D + 1, :, :], AF.Exp, scale=-1.0
            )
            # partition_broadcast's gpsimd ucode reads the source with Q7
            # core 0, so the reciprocal row is DMA'd to partition 0 first.
            nc.sync.dma_start(rc[0:1, :, :], rc[D : D + 1, :, :])
            rbb = epool.tile([D, 2, TI], FP32, tag="rbb")
            nc.gpsimd.partition_broadcast(rbb[:], rc[0:1, :, :])
            nc.vector.tensor_tensor(
                yT_sb[0:D, pr, islice], osb[0:D, 0, :], rbb[:, 0, :], MULT
            )
            tmp = epool.tile([D, TI], BF16, tag="tmp")
            nc.vector.tensor_tensor(tmp[:], osb[0:D, 1, :], rbb[:, 1, :], MULT)
            # odd head's y^T lives on partitions 64-127: cross-partition
            # move must go through DMA
            nc.sync.dma_start(yT_sb[D:P, pr, islice], tmp[:])

        def cproj2_items(tt, oc, grp):
            """Last-chunk c_proj: one 2-pair partial (K=256) so the tiles
            can drain as soon as their two pairs finish, instead of one
            big all-pairs burst after the final epilogue."""
            state = {}

            def mk(i):
                pr = 2 * grp + i

                def f():
                    if i == 0:
                        state["t"] = cp_ps.tile([P, TI], FP32, tag="cp", name="cp2")
                    nc.tensor.matmul(
                        state["t"][:],
                        yT_sb[:, pr, tt * P : (tt + 1) * P],
                        wp_sb[:, pr, oc * TI : (oc + 1) * TI],
                        start=(i == 0),
                        stop=(i == 1),
                    )
                return f

            items = [(None, mk(i)) for i in range(2)]
            tl = tt - 4 * (NIT - 1)

            def store():
                ob = opool.tile([P, TI], BF16, tag="ob", name="ob2")
                nc.vector.tensor_copy(ob[:], state["t"][:])
                nc.sync.dma_start(
                    out2[grp, tl * P : (tl + 1) * P, oc * TI : (oc + 1) * TI], ob[:]
                )

            items.append((None, store))
            return items

        # ---- prologue compute: pair 0's chunk-0 q/k and V tile 0 ----
        for key, f in qkv_group_items(0, 1, 0) + qkv_group_items(0, 0, 0) + v_tile_items(0):
            f()
            if key is not None:
                done.add(key)

        # phase-0 filler: remaining chunk-0 tiles (all ready once their
        # DMAs land), then chunk-1 work
        for tt in (1, 2):
            queue += v_tile_items(tt)
        queue += qkv_group_items(1, 1, 0)
        queue += qkv_group_items(1, 0, 0)
        queue += v_tile_items(3)
        queue += qkv_group_items(2, 1, 0)
        queue += qkv_group_items(2, 0, 0)
        queue += qkv_group_items(3, 1, 0)
        queue += qkv_group_items(3, 0, 0)

        pending = [None]
        for it in range(NIT):
            njt = 4 * (it + 1)
            if it + 1 < NIT:
                for pr in range(JQ):
                    queue += qkv_group_items(pr, 1, it + 1)
                    queue += qkv_group_items(pr, 0, it + 1)
                for tt in range(4 * (it + 1), 4 * (it + 2)):
                    queue += v_tile_items(tt)
            if it > 0:
                for tt in range(4 * (it - 1), 4 * it):
                    for oc in range(NOC):
                        queue += cproj_items(tt, oc)
            for pr in range(JQ):
                # emit the new unit's first scores+exp, THEN the previous
                # unit's deferred normalize chain (its Ln/Exp slot in the
                # ScalarE FIFO lands after this exp, so the PV-completion
                # wait overlaps exp execution), THEN allocate the single
                # PSUM accumulator (WAR on the deferred Ln is tracked)
                p2_0, lo_0 = scores_part(pr, it, 0)
                if pending[0] is not None:
                    epi_rest(*pending[0])
                    pending[0] = None
                po = o_ps.tile([P, 2, TI], FP32, tag="po")
                pv_part(pr, 0, njt, po, p2_0, lo_0)
                if it == 0:
                    drain(7)
                else:
                    n = max(2, -(-len(queue) // max(1, slots_left[0])))
                    drain(min(n, 8))
                slots_left[0] -= 1
                for jt in range(1, njt):
                    slot(pr, it, jt, njt, po)
                    if it == 0:
                        drain(7)
                    else:
                        n = max(2, -(-len(queue) // max(1, slots_left[0])))
                        drain(min(n, 8))
                    slots_left[0] -= 1
                osb = epi_copy(po)
                pending[0] = (pr, it, po, osb)
                drain(3 if it > 0 else 7)
                slots_left[0] -= 1
                if it == NIT - 1 and pr == 1:
                    # pairs 0/1 done with the last chunk: their c_proj
                    # partial drains during the remaining two units
                    for tt in range(4 * (NIT - 1), 4 * NIT):
                        for oc in range(NOC):
                            queue += cproj2_items(tt, oc, 0)

        # tail: final epilogue, then the last 2-pair c_proj partial
        epi_rest(*pending[0])
        for tt in range(4 * (NIT - 1), 4 * NIT):
            for oc in range(NOC):
                queue += cproj2_items(tt, oc, 1)
        drain(len(queue))


def build_nc():
    nc = bacc.Bacc("TRN2", target_bir_lowering=False, debug=False)
    io = {
        "xT": nc.dram_tensor("xT", [C, T], BF16, kind="ExternalInput").ap(),
        "wqkv": nc.dram_tensor("wqkv", [C, 3 * CL], BF16, kind="ExternalInput").ap(),
        "bqk": nc.dram_tensor("bqk", [P, 2 * JQ], FP32, kind="ExternalInput").ap(),
        "bv": nc.dram_tensor("bv", [1, CL], FP32, kind="ExternalInput").ap(),
        "wp": nc.dram_tensor("wp", [CL, C], BF16, kind="ExternalInput").ap(),
        "mtri": nc.dram_tensor("mtri", [P, P], FP32, kind="ExternalInput").ap(),
        # one partial [T, C] per core (c_proj contracted over this core's
        # 8 heads); the host sums the two per-batch partials in fp32
        "out": nc.dram_tensor("out", [T, C], BF16, kind="ExternalOutput").ap(),
        # last-chunk 2-pair partials: [grp, t - (T-512), c]
        "out2": nc.dram_tensor("out2", [2, TI, C], BF16, kind="ExternalOutput").ap(),
    }
    with tile.TileContext(nc) as tc:
        _emit(tc, io)
    # The act-table-load pass assigns each activation the FIRST table set
    # containing its function, so Exp->'exp_and_others' and
    # Ln->'natural_log' alternate (a 1.3us ACT_TABLE_LOAD per switch).
    # Restrict the choice to 'natural_log_exp_and_others' (which holds
    # every function this kernel uses) so exactly one table load is
    # emitted.  Set ids stay aligned with act_info.json because the dict
    # keeps all entries in order.
    orig_tables = bacc.get_activation_tables

    def _combined_only(arch):
        t = orig_tables(arch)
        return {
            name: (funcs if name == "natural_log_exp_and_others" else set())
            for name, funcs in t.items()
        }

    bacc.get_activation_tables = _combined_only
    try:
        nc.compile()
    finally:
        bacc.get_activation_tables = orig_tables
    return nc


def make_in_maps(x, W_attn, b_attn, W_proj):
    """Per-core input dicts: core = 2*batch + head_group."""
    bf = ml_dtypes.bfloat16
    scale = np.float32(1.0 / math.sqrt(D))
    mtri = np.where(
        np.arange(P)[None, :] < np.arange(P)[:, None],
        np.float32(NEG),
        np.float32(0.0),
    ).astype(np.float32)
    in_maps = []
    for core in range(NCORES):
        b, g = divmod(core, 2)
        hs = slice(g * CL, (g + 1) * CL)
        wq = (W_attn[:, 0:C][:, hs] * scale).astype(bf)
        wk = W_attn[:, C : 2 * C][:, hs].astype(bf)
        wv = W_attn[:, 2 * C : 3 * C][:, hs].astype(bf)
        bq = (b_attn[0:C][hs] * scale).astype(np.float32)
        bk = b_attn[C : 2 * C][hs].astype(np.float32)
        bv = b_attn[2 * C : 3 * C][hs].astype(np.float32)
        in_maps.append(
            {
                "xT": np.ascontiguousarray(x[b].T).astype(bf),
                "wqkv": np.ascontiguousarray(np.concatenate([wq, wk, wv], axis=1)),
                "bqk": np.ascontiguousarray(
                    np.concatenate([bq, bk]).reshape(2 * JQ, P).T
                ),
                "bv": bv.reshape(1, CL),
                "wp": np.ascontiguousarray(W_proj[hs, :]).astype(bf),
                "mtri": mtri,
            }
        )
    return in_maps


def combine_outputs(results, b_proj):
    out = np.empty((B, T, C), np.float32)
    t0 = T - TI
    for b in range(B):
        acc = results[2 * b]["out"].astype(np.float32)
        acc = acc + results[2 * b + 1]["out"].astype(np.float32)
        for g in range(2):
            acc[t0:] += results[2 * b]["out2"][g].astype(np.float32)
            acc[t0:] += results[2 * b + 1]["out2"][g].astype(np.float32)
        acc += b_proj.astype(np.float32)[None, :]
        out[b] = acc
    return out


def _mask_is_causal(mask):
    if mask.shape != (B, T, T):
        return False
    tril = np.tril(np.ones((T, T), np.float32))
    return all(np.array_equal(np.asarray(mask[b]), tril) for b in range(B))


def _numpy_fallback(x, mask, W_attn, b_attn, W_proj, b_proj):
    # generic-mask fallback (never hit for the causal reference inputs)
    out = np.empty((B, T, C), np.float32)
    for b in range(B):
        qkv = x[b] @ W_attn + b_attn
        q, k, v = np.split(qkv, 3, axis=-1)
        q = q.reshape(T, H, D)
        k = k.reshape(T, H, D)
        v = v.reshape(T, H, D)
        y = np.empty((T, H, D), np.float32)
        for h in range(H):
            s = (q[:, h] @ k[:, h].T) / math.sqrt(D)
            s = s + NEG * (1.0 - mask[b])
            s = s - s.max(-1, keepdims=True)
            p = np.exp(s)
            p /= p.sum(-1, keepdims=True)
            y[:, h] = p @ v[:, h]
        out[b] = y.reshape(T, C) @ W_proj + b_proj
    return out


_NC = None


def kernel(x, mask, W_attn, b_attn, W_proj, b_proj):
    global _NC
    x = np.asarray(x, dtype=np.float32)
    mask = np.asarray(mask)
    W_attn = np.asarray(W_attn, dtype=np.float32)
    b_attn = np.asarray(b_attn, dtype=np.float32)
    W_proj = np.asarray(W_proj, dtype=np.float32)
    b_proj = np.asarray(b_proj, dtype=np.float32)

    if not _mask_is_causal(mask):
        return _numpy_fallback(x, mask, W_attn, b_attn, W_proj, b_proj)

    if _NC is None:
        _NC = build_nc()
    in_maps = make_in_maps(x, W_attn, b_attn, W_proj)
    res = run_bass_kernel_spmd(_NC, in_maps, core_ids=list(range(NCORES)))
    return combine_outputs(res.results, b_proj)


# revision 7
# speedup vs baseline: 1.2196x; 1.2196x over previous
"""Causal self-attention (B=4, T=2048, C=1024, H=16) on 8 TRN2 NeuronCores.

Sharding: core = (batch b, head-group g) with b in 0..3, g in 0..1.
Each core handles one batch element and 8 of the 16 heads (tensor-parallel
split of the QKV / proj weights).  Each core produces one partial [T, C]
output (c_proj contracted over its 8 heads, accumulated in PSUM across the
4 head pairs); the host sums the two per-batch partials and adds b_proj.

Device layout (per core) -- everything SBUF-resident, bf16 matmul inputs:
  xT    [C, T]        x[b]^T, host-transposed + bf16-cast
  wqkv  [C, 3*CL]     W_attn column slice for this head group (q scaled by
                      1/sqrt(D) on host), bf16
  qkvT = wqkv.T @ xT computed as [ch, t] tiles (q^T, k^T); v computed in
  natural [t, d] orientation as xT.T @ wv.
  Attention per head pair: S^T[j, i] = k^T.T q^T (contraction d=64, two
  heads packed on PE row-groups 0-63 / 64-127 running concurrently),
  structural causal masking (only lower-triangular j-tiles computed;
  diagonal tiles get an additive -1e9 triangle constant), exp on ScalarE
  (no max subtraction -- scores are O(6)), P^T @ V' on PE with
  V' = [V | ones] so row 64 of the accumulator is the softmax denominator.

  Schedule is query-chunk-outer (it = chunk of 512 queries): for each it,
  the 4 head pairs run their attention for that chunk; c_proj for chunk
  it-1 (contraction over all 4 pairs, K=512 accumulated in PSUM) drains as
  PE filler between attention slots, together with the next chunk's q/k
  projections and V tiles.  This keeps ScalarE (the exp stream, ~1.15us
  per slot vs ~0.64us of PE work) saturated while the PE runs projection
  matmuls in the gaps, and keeps the PE HAM-warm (no >3us idle windows).

  Normalization: ln(den) read directly from the PSUM accumulator row 64,
  rec = exp(-ln(den)) (Ln and Exp share one ACT table set), DMA of the rec
  row to partition 0, gpsimd partition-broadcast to 64 partitions, then
  one DVE multiply per head; the unnormalized y rows are copied to SBUF
  right after the last PV matmul so the single PSUM accumulator can be
  reused by the next head pair immediately.
"""

import math

import ml_dtypes
import numpy as np

import concourse.bass as bass
import concourse.tile as tile
from concourse import bacc, mybir
from concourse.bass_utils import run_bass_kernel_spmd

# problem shape (hardcoded per the task contract)
B, T, C, H = 4, 2048, 1024, 16
D = C // H            # 64 head dim
NCORES = 8
HL = H // 2           # heads per core
CL = HL * D           # 512 local channels per core
NEG = -1.0e9

P = 128               # SBUF partitions
TI = 512              # query chunk (matmul moving dim)
TJ = 128              # key tile
CT = C // P           # 8 contraction tiles for the projections
NTT = T // P          # 16 t-tiles of 128
NIT = T // TI         # 4 query chunks
JQ = CL // P          # 4 channel tiles for q (and for k, and for y)
KC = CL // P          # 4 channel tiles in c_proj contraction
NOC = C // TI         # 2 output-column tiles in c_proj

FP32 = mybir.dt.float32
BF16 = mybir.dt.bfloat16
AF = mybir.ActivationFunctionType
ADD = mybir.AluOpType.add
MULT = mybir.AluOpType.mult


def _emit(tc, io):
    nc = tc.nc
    xT, wqkv, bqk, bv, wp, mtri, out, out2 = (
        io["xT"], io["wqkv"], io["bqk"], io["bv"], io["wp"], io["mtri"],
        io["out"], io["out2"],
    )

    with (
        tc.tile_pool(name="const", bufs=1) as cpool,
        tc.tile_pool(name="work", bufs=4) as wpool,
        tc.tile_pool(name="epi", bufs=2) as epool,
        tc.tile_pool(name="outp", bufs=3) as opool,
        tc.tile_pool(name="ps", bufs=2, space="PSUM") as s_ps,
        tc.tile_pool(name="po", bufs=1, space="PSUM") as o_ps,
        tc.tile_pool(name="mm", bufs=1, space="PSUM") as mm_ps,
        tc.tile_pool(name="cp", bufs=1, space="PSUM") as cp_ps,
    ):
        # persistent SBUF tensors
        xT_sb = cpool.tile([P, CT, T], BF16)
        wqkv_sb = cpool.tile([P, CT, 3 * CL], BF16)
        qT_sb = cpool.tile([P, JQ, T], BF16)
        kT_sb = cpool.tile([P, JQ, T], BF16)
        v_sb = cpool.tile([P, NTT, HL, D + 1], BF16)
        yT_sb = cpool.tile([P, JQ, T], BF16)
        wp_sb = cpool.tile([P, KC, C], BF16)
        mtri_sb = cpool.tile([P, P], FP32)
        bqk_sb = cpool.tile([P, 2 * JQ], FP32)
        bv_sb = cpool.tile([1, CL], FP32)
        bvb_sb = cpool.tile([P, CL], FP32)

        xT_d = xT.rearrange("(o p) t -> p o t", p=P)
        wqkv_d = wqkv.rearrange("(o p) j -> p o j", p=P)

        # prologue DMAs in first-compute order: tiny constants, then the
        # pair-0 q/k weight column slices + x^T chunk 0 (first matmuls),
        # v weights, then the rest interleaved so each pair / chunk lands
        # just before its first use.  Total input is ~8.4MB = ~23us at HBM
        # rate; fine-grained ordering lets compute start at ~4us.
        # Dummy first activation: walrus inserts the ACT_TABLE_LOAD right
        # before the first ACTIVATE in the ScalarE program, so without
        # this the ~2.7us table load+drain lands behind the first exp's
        # semaphore wait, on the critical path.  The [1,2] corner of mtri
        # is overwritten by its DMA right after (WAW tracked).
        nc.scalar.activation(mtri_sb[0:1, 0:2], bvb_sb[0:1, 0:2], AF.Exp)

        # Input loads spread over the three DGE rings so the first-needed
        # tiles land fastest: tiny constants on the ACT ring, pair-0
        # weights + w_v + x^T chunk 1 on the sync ring, x^T chunk 0 first
        # on the gpsimd SWDGE ring followed by the remaining weights.
        # The sync ring drains by ~14us so the epilogue row DMAs are not
        # queued behind bulk input.
        def dma_wslice(eng, which, pr):
            j0 = which * CL + pr * P
            eng.dma_start(wqkv_sb[:, :, j0 : j0 + P], wqkv_d[:, :, j0 : j0 + P])

        nc.scalar.dma_start(bqk_sb[:], bqk[:])
        nc.scalar.dma_start(bv_sb[:], bv[:])
        nc.scalar.dma_start(mtri_sb[:], mtri[:])
        dma_wslice(nc.sync, 1, 0)  # w_k pair 0
        dma_wslice(nc.sync, 0, 0)  # w_q pair 0
        nc.sync.dma_start(wqkv_sb[:, :, 2 * CL :], wqkv_d[:, :, 2 * CL :])  # w_v
        nc.sync.dma_start(xT_sb[:, :, TI : 2 * TI], xT_d[:, :, TI : 2 * TI])
        nc.gpsimd.dma_start(xT_sb[:, :, 0:TI], xT_d[:, :, 0:TI])
        nc.gpsimd.partition_broadcast(bvb_sb[:], bv_sb[:])
        dma_wslice(nc.gpsimd, 1, 1)
        dma_wslice(nc.gpsimd, 0, 1)
        dma_wslice(nc.gpsimd, 1, 2)
        dma_wslice(nc.gpsimd, 0, 2)
        dma_wslice(nc.gpsimd, 1, 3)
        dma_wslice(nc.gpsimd, 0, 3)
        nc.gpsimd.dma_start(xT_sb[:, :, 2 * TI : 3 * TI], xT_d[:, :, 2 * TI : 3 * TI])
        nc.gpsimd.dma_start(wp_sb[:], wp.rearrange("(o p) j -> p o j", p=P))
        nc.gpsimd.dma_start(xT_sb[:, :, 3 * TI :], xT_d[:, :, 3 * TI :])
        # ones column of V' (softmax denominator accumulator)
        nc.vector.memset(v_sb[:, :, :, D : D + 1], 1.0)

        wv = wqkv_sb[:, :, 2 * CL : 3 * CL]

        # ---- PE filler: projection / c_proj work queued as ~1-matmul
        # items and drained between attention slots.  Items carry a key on
        # their last (finalizing) op so attention slots can force-drain
        # their producers before being emitted -- the Tile framework only
        # tracks dependencies in emission order, so a consumer emitted
        # before its producer would silently read stale SBUF. ----
        queue = []
        done = set()
        slots_left = [sum(4 * (it + 1) for it in range(NIT)) * JQ + 4 * NIT]

        def drain(n):
            for _ in range(min(n, len(queue))):
                key, f = queue.pop(0)
                f()
                if key is not None:
                    done.add(key)

        def drain_until(key):
            while key not in done:
                assert queue, f"filler item {key} was never enqueued"
                k, f = queue.pop(0)
                f()
                if k is not None:
                    done.add(k)

        def v_tile_items(tt):
            """V in natural [t, d] orientation: V = xT.T @ wv, one t-tile."""
            state = {}

            def mk(o):
                def f():
                    if o == 0:
                        state["t"] = mm_ps.tile([P, CL], FP32, tag="mm", name="vmm")
                    nc.tensor.matmul(
                        state["t"][:],
                        xT_sb[:, o, tt * P : (tt + 1) * P],
                        wv[:, o, :],
                        start=(o == 0),
                        stop=(o == CT - 1),
                    )
                return f

            items = [(None, mk(o)) for o in range(CT)]

            def bias():
                nc.vector.tensor_tensor(
                    v_sb[:, tt, :, 0:D],
                    state["t"].rearrange("p (h d) -> p h d", h=HL),
                    bvb_sb.rearrange("p (h d) -> p h d", h=HL),
                    ADD,
                )

            items.append((("v", tt), bias))
            return items

        def qkv_group_items(pr, which, tch):
            """One [128-ch, 512-t] q^T (which=0) or k^T (which=1) tile."""
            jt = which * JQ + pr
            dst = qT_sb if which == 0 else kT_sb
            state = {}

            def mk(o):
                def f():
                    if o == 0:
                        state["t"] = mm_ps.tile([P, TI], FP32, tag="mm", name="qkmm")
                    nc.tensor.matmul(
                        state["t"][:],
                        wqkv_sb[:, o, jt * P : (jt + 1) * P],
                        xT_sb[:, o, tch * TI : (tch + 1) * TI],
                        start=(o == 0),
                        stop=(o == CT - 1),
                    )
                return f

            items = [(None, mk(o)) for o in range(CT)]

            def bias():
                nc.vector.tensor_scalar_add(
                    dst[:, pr, tch * TI : (tch + 1) * TI],
                    state["t"][:],
                    bqk_sb[:, jt : jt + 1],
                )

            items.append((("qkv", which, pr, tch), bias))
            return items

        def cproj_items(tt, oc):
            """One [128-t, 512-c] c_proj output tile, K=512 accumulated in
            PSUM over the 4 head pairs, stored to DRAM as bf16."""
            state = {}

            def mk(pr):
                def f():
                    if pr == 0:
                        state["t"] = cp_ps.tile([P, TI], FP32, tag="cp", name="cpmm")
                    nc.tensor.matmul(
                        state["t"][:],
                        yT_sb[:, pr, tt * P : (tt + 1) * P],
                        wp_sb[:, pr, oc * TI : (oc + 1) * TI],
                        start=(pr == 0),
                        stop=(pr == JQ - 1),
                    )
                return f

            items = [(None, mk(pr)) for pr in range(JQ)]

            def store():
                ob = opool.tile([P, TI], BF16, tag="ob", name="ob")
                nc.vector.tensor_copy(ob[:], state["t"][:])
                nc.sync.dma_start(
                    out[tt * P : (tt + 1) * P, oc * TI : (oc + 1) * TI], ob[:]
                )

            items.append((None, store))
            return items

        # ---- attention slot + unit epilogue ----
        def scores_part(pr, it, jt):
            # force-emit this slot's producers (emission order = the only
            # dependency order Tile sees)
            drain_until(("qkv", 0, pr, it))
            drain_until(("qkv", 1, pr, jt * TJ // TI))
            delta = jt * TJ - it * TI
            lo = max(delta, 0)
            ps = s_ps.tile([P, 2, TI], FP32, tag="ps")
            # S^T = k^T.T @ q^T, contraction d=64; the two heads of the
            # pair sit on PE row groups 0-63 / 64-127 and run concurrently.
            nc.tensor.matmul(
                ps[:, 0, lo:TI],
                kT_sb[0:D, pr, jt * TJ : (jt + 1) * TJ],
                qT_sb[0:D, pr, it * TI + lo : (it + 1) * TI],
                start=True,
                stop=True,
            )
            nc.tensor.matmul(
                ps[:, 1, lo:TI],
                kT_sb[D:P, pr, jt * TJ : (jt + 1) * TJ],
                qT_sb[D:P, pr, it * TI + lo : (it + 1) * TI],
                start=True,
                stop=True,
                tile_position=(D, 0),
            )
            if delta >= 0:  # diagonal tile: strict upper triangle -> -1e9
                nc.vector.tensor_tensor(
                    ps[:, :, delta : delta + TJ],
                    ps[:, :, delta : delta + TJ],
                    mtri_sb[:, None, :].to_broadcast((P, 2, TJ)),
                    ADD,
                )
            p2 = wpool.tile([P, 2, TI], BF16, tag="p2")
            # columns [0:lo) are fully masked and the PV matmuls only read
            # [lo:], so exp is restricted and no memset is needed
            if lo > 0:
                nc.scalar.activation(p2[:, :, lo:TI], ps[:, :, lo:TI], AF.Exp)
            else:
                nc.scalar.activation(p2[:], ps[:], AF.Exp)
            return p2, lo

        def pv_part(pr, jt, njt, po, p2, lo):
            drain_until(("v", jt))
            first, last = (jt == 0), (jt == njt - 1)
            nc.tensor.matmul(
                po[0 : D + 1, 0, lo:TI],
                v_sb[:, jt, 2 * pr, :],
                p2[:, 0, lo:TI],
                start=first,
                stop=last,
            )
            nc.tensor.matmul(
                po[0 : D + 1, 1, lo:TI],
                v_sb[:, jt, 2 * pr + 1, :],
                p2[:, 1, lo:TI],
                start=first,
                stop=last,
            )

        def slot(pr, it, jt, njt, po):
            p2, lo = scores_part(pr, it, jt)
            pv_part(pr, jt, njt, po, p2, lo)

        def epi_copy(po):
            # free the PSUM accumulator fast: plain copy of y-hat + den
            osb = epool.tile([D + 1, 2, TI], FP32, tag="osb")
            nc.vector.tensor_copy(osb[:], po[0 : D + 1, :, :])
            return osb

        def epi_rest(pr, it, po, osb):
            """Normalize the pair's y^T rows for this it-chunk.  Emitted
            after the NEXT unit's first exp so the Ln/Exp pair does not
            stall the ScalarE exp stream at the unit boundary (the Ln
            waits on this unit's last PV matmul)."""
            islice = slice(it * TI, (it + 1) * TI)
            # rec = exp(-ln(den)) -- Ln/Exp share one ACT table set, and
            # Ln reads the denominator row straight from PSUM (both
            # parities in one [1, 1024] pass: the two po banks are
            # adjacent inside the single [128, 2, 512] accumulator tile).
            rl = epool.tile([D + 1, 2, TI], FP32, tag="rl")
            nc.scalar.activation(rl[D : D + 1, :, :], po[D : D + 1, :, :], AF.Ln)
            rc = epool.tile([D + 1, 2, TI], FP32, tag="rc")
            nc.scalar.activation(
                rc[D : D + 1, :, :], rl[D : D + 1, :, :], AF.Exp, scale=-1.0
            )
            # partition_broadcast's gpsimd ucode reads the source with Q7
            # core 0, so the reciprocal row is DMA'd to partition 0 first.
            nc.sync.dma_start(rc[0:1, :, :], rc[D : D + 1, :, :])
            rbb = epool.tile([D, 2, TI], FP32, tag="rbb")
            nc.gpsimd.partition_broadcast(rbb[:], rc[0:1, :, :])
            nc.vector.tensor_tensor(
                yT_sb[0:D, pr, islice], osb[0:D, 0, :], rbb[:, 0, :], MULT
            )
            tmp = epool.tile([D, TI], BF16, tag="tmp")
            nc.vector.tensor_tensor(tmp[:], osb[0:D, 1, :], rbb[:, 1, :], MULT)
            # odd head's y^T lives on partitions 64-127: cross-partition
            # move must go through DMA
            nc.sync.dma_start(yT_sb[D:P, pr, islice], tmp[:])

        def cproj2_items(tt, oc, grp):
            """Last-chunk c_proj: one 2-pair partial (K=256) so the tiles
            can drain as soon as their two pairs finish, instead of one
            big all-pairs burst after the final epilogue."""
            state = {}

            # alternate the single-buffered cp/mm PSUM pools and the
            # DVE/ScalarE copy engines across tiles so consecutive tiles
            # pipeline instead of serializing on the WAR chain at the tail
            alt = (tt * NOC + oc) % 2

            def mk(i):
                pr = 2 * grp + i

                def f():
                    if i == 0:
                        pool, tg = (cp_ps, "cp") if alt == 0 else (mm_ps, "mm")
                        state["t"] = pool.tile([P, TI], FP32, tag=tg, name="cp2")
                    nc.tensor.matmul(
                        state["t"][:],
                        yT_sb[:, pr, tt * P : (tt + 1) * P],
                        wp_sb[:, pr, oc * TI : (oc + 1) * TI],
                        start=(i == 0),
                        stop=(i == 1),
                    )
                return f

            items = [(None, mk(i)) for i in range(2)]
            tl = tt - 4 * (NIT - 1)

            def store():
                ob = opool.tile([P, TI], BF16, tag="ob", name="ob2")
                if alt == 0:
                    nc.vector.tensor_copy(ob[:], state["t"][:])
                else:
                    nc.scalar.copy(ob[:], state["t"][:])
                nc.sync.dma_start(
                    out2[grp, tl * P : (tl + 1) * P, oc * TI : (oc + 1) * TI], ob[:]
                )

            items.append((None, store))
            return items

        # ---- prologue compute: pair 0's chunk-0 q/k and V tile 0 ----
        for key, f in qkv_group_items(0, 1, 0) + qkv_group_items(0, 0, 0) + v_tile_items(0):
            f()
            if key is not None:
                done.add(key)

        # phase-0 filler: remaining chunk-0 tiles (all ready once their
        # DMAs land), then chunk-1 work
        for tt in (1, 2):
            queue += v_tile_items(tt)
        queue += qkv_group_items(1, 1, 0)
        queue += qkv_group_items(1, 0, 0)
        queue += v_tile_items(3)
        queue += qkv_group_items(2, 1, 0)
        queue += qkv_group_items(2, 0, 0)
        queue += qkv_group_items(3, 1, 0)
        queue += qkv_group_items(3, 0, 0)

        pending = [None]
        for it in range(NIT):
            njt = 4 * (it + 1)
            if it + 1 < NIT:
                for pr in range(JQ):
                    queue += qkv_group_items(pr, 1, it + 1)
                    queue += qkv_group_items(pr, 0, it + 1)
                for tt in range(4 * (it + 1), 4 * (it + 2)):
                    queue += v_tile_items(tt)
            if it > 0:
                for tt in range(4 * (it - 1), 4 * it):
                    for oc in range(NOC):
                        queue += cproj_items(tt, oc)
            for pr in range(JQ):
                # emit the new unit's first scores+exp, THEN the previous
                # unit's deferred normalize chain (its Ln/Exp slot in the
                # ScalarE FIFO lands after this exp, so the PV-completion
                # wait overlaps exp execution), THEN allocate the single
                # PSUM accumulator (WAR on the deferred Ln is tracked)
                p2_0, lo_0 = scores_part(pr, it, 0)
                if pending[0] is not None:
                    epi_rest(*pending[0])
                    pending[0] = None
                po = o_ps.tile([P, 2, TI], FP32, tag="po")
                pv_part(pr, 0, njt, po, p2_0, lo_0)
                if it == 0:
                    drain(7)
                else:
                    n = max(2, -(-len(queue) // max(1, slots_left[0])))
                    drain(min(n, 8))
                slots_left[0] -= 1
                for jt in range(1, njt):
                    slot(pr, it, jt, njt, po)
                    if it == 0:
                        drain(7)
                    else:
                        n = max(2, -(-len(queue) // max(1, slots_left[0])))
                        drain(min(n, 8))
                    slots_left[0] -= 1
                osb = epi_copy(po)
                pending[0] = (pr, it, po, osb)
                drain(3 if it > 0 else 7)
                slots_left[0] -= 1
                if it == NIT - 1 and pr == 1:
                    # pairs 0/1 done with the last chunk: their c_proj
                    # partial drains during the remaining two units
                    for tt in range(4 * (NIT - 1), 4 * NIT):
                        for oc in range(NOC):
                            queue += cproj2_items(tt, oc, 0)

        # tail: final epilogue, then the last 2-pair c_proj partial
        epi_rest(*pending[0])
        for tt in range(4 * (NIT - 1), 4 * NIT):
            for oc in range(NOC):
                queue += cproj2_items(tt, oc, 1)
        drain(len(queue))


def build_nc():
    nc = bacc.Bacc("TRN2", target_bir_lowering=False, debug=False)
    io = {
        "xT": nc.dram_tensor("xT", [C, T], BF16, kind="ExternalInput").ap(),
        "wqkv": nc.dram_tensor("wqkv", [C, 3 * CL], BF16, kind="ExternalInput").ap(),
        "bqk": nc.dram_tensor("bqk", [P, 2 * JQ], FP32, kind="ExternalInput").ap(),
        "bv": nc.dram_tensor("bv", [1, CL], FP32, kind="ExternalInput").ap(),
        "wp": nc.dram_tensor("wp", [CL, C], BF16, kind="ExternalInput").ap(),
        "mtri": nc.dram_tensor("mtri", [P, P], FP32, kind="ExternalInput").ap(),
        # one partial [T, C] per core (c_proj contracted over this core's
        # 8 heads); the host sums the two per-batch partials in fp32
        "out": nc.dram_tensor("out", [T, C], BF16, kind="ExternalOutput").ap(),
        # last-chunk 2-pair partials: [grp, t - (T-512), c]
        "out2": nc.dram_tensor("out2", [2, TI, C], BF16, kind="ExternalOutput").ap(),
    }
    with tile.TileContext(nc) as tc:
        _emit(tc, io)
    # The act-table-load pass assigns each activation the FIRST table set
    # containing its function, so Exp->'exp_and_others' and
    # Ln->'natural_log' alternate (a 1.3us ACT_TABLE_LOAD per switch).
    # Restrict the choice to 'natural_log_exp_and_others' (which holds
    # every function this kernel uses) so exactly one table load is
    # emitted.  Set ids stay aligned with act_info.json because the dict
    # keeps all entries in order.
    orig_tables = bacc.get_activation_tables

    def _combined_only(arch):
        t = orig_tables(arch)
        return {
            name: (funcs if name == "natural_log_exp_and_others" else set())
            for name, funcs in t.items()
        }

    bacc.get_activation_tables = _combined_only
    try:
        nc.compile()
    finally:
        bacc.get_activation_tables = orig_tables
    return nc


def make_in_maps(x, W_attn, b_attn, W_proj):
    """Per-core input dicts: core = 2*batch + head_group."""
    bf = ml_dtypes.bfloat16
    scale = np.float32(1.0 / math.sqrt(D))
    mtri = np.where(
        np.arange(P)[None, :] < np.arange(P)[:, None],
        np.float32(NEG),
        np.float32(0.0),
    ).astype(np.float32)
    in_maps = []
    for core in range(NCORES):
        b, g = divmod(core, 2)
        hs = slice(g * CL, (g + 1) * CL)
        wq = (W_attn[:, 0:C][:, hs] * scale).astype(bf)
        wk = W_attn[:, C : 2 * C][:, hs].astype(bf)
        wv = W_attn[:, 2 * C : 3 * C][:, hs].astype(bf)
        bq = (b_attn[0:C][hs] * scale).astype(np.float32)
        bk = b_attn[C : 2 * C][hs].astype(np.float32)
        bv = b_attn[2 * C : 3 * C][hs].astype(np.float32)
        in_maps.append(
            {
                "xT": np.ascontiguousarray(x[b].T).astype(bf),
                "wqkv": np.ascontiguousarray(np.concatenate([wq, wk, wv], axis=1)),
                "bqk": np.ascontiguousarray(
                    np.concatenate([bq, bk]).reshape(2 * JQ, P).T
                ),
                "bv": bv.reshape(1, CL),
                "wp": np.ascontiguousarray(W_proj[hs, :]).astype(bf),
                "mtri": mtri,
            }
        )
    return in_maps


def combine_outputs(results, b_proj):
    out = np.empty((B, T, C), np.float32)
    t0 = T - TI
    for b in range(B):
        acc = results[2 * b]["out"].astype(np.float32)
        acc = acc + results[2 * b + 1]["out"].astype(np.float32)
        for g in range(2):
            acc[t0:] += results[2 * b]["out2"][g].astype(np.float32)
            acc[t0:] += results[2 * b + 1]["out2"][g].astype(np.float32)
        acc += b_proj.astype(np.float32)[None, :]
        out[b] = acc
    return out


def _mask_is_causal(mask):
    if mask.shape != (B, T, T):
        return False
    tril = np.tril(np.ones((T, T), np.float32))
    return all(np.array_equal(np.asarray(mask[b]), tril) for b in range(B))


def _numpy_fallback(x, mask, W_attn, b_attn, W_proj, b_proj):
    # generic-mask fallback (never hit for the causal reference inputs)
    out = np.empty((B, T, C), np.float32)
    for b in range(B):
        qkv = x[b] @ W_attn + b_attn
        q, k, v = np.split(qkv, 3, axis=-1)
        q = q.reshape(T, H, D)
        k = k.reshape(T, H, D)
        v = v.reshape(T, H, D)
        y = np.empty((T, H, D), np.float32)
        for h in range(H):
            s = (q[:, h] @ k[:, h].T) / math.sqrt(D)
            s = s + NEG * (1.0 - mask[b])
            s = s - s.max(-1, keepdims=True)
            p = np.exp(s)
            p /= p.sum(-1, keepdims=True)
            y[:, h] = p @ v[:, h]
        out[b] = y.reshape(T, C) @ W_proj + b_proj
    return out


_NC = None


def kernel(x, mask, W_attn, b_attn, W_proj, b_proj):
    global _NC
    x = np.asarray(x, dtype=np.float32)
    mask = np.asarray(mask)
    W_attn = np.asarray(W_attn, dtype=np.float32)
    b_attn = np.asarray(b_attn, dtype=np.float32)
    W_proj = np.asarray(W_proj, dtype=np.float32)
    b_proj = np.asarray(b_proj, dtype=np.float32)

    if not _mask_is_causal(mask):
        return _numpy_fallback(x, mask, W_attn, b_attn, W_proj, b_proj)

    if _NC is None:
        _NC = build_nc()
    in_maps = make_in_maps(x, W_attn, b_attn, W_proj)
    res = run_bass_kernel_spmd(_NC, in_maps, core_ids=list(range(NCORES)))
    return combine_outputs(res.results, b_proj)
